# revision 18
# baseline (speedup 1.0000x reference)
"""Trainium2 Bass kernel for nn_DualAttention (S=2048, B=16, H2=2048, V=1024).

Computation (per the reference):
    sum_w = hidden @ Ww + bw + z @ Wz + bz + w_a*0.5        [S, B, V]
    u     = tanh(sum_w) @ Vw + vb                            [S, B, 1]
    out   = softmax(u, axis=0)                               [S, B, 1]

Strategy
--------
Data-parallel over batch: 16 batches -> 2 per NeuronCore (8 cores).
Host-side prep per core:
  * concat hidden/z along the hidden axis -> X [ROWS=4096, H=4096]
    (rows are b-major: row = b_local*2048 + s)
  * transpose + pack X^T into [NRB, P, NK, RB] (per-partition contiguous
    chunks for each (rowblock, k-group) DMA), cast to bf16
  * W = concat([Ww, Wz], 0) [H, V], reordered into per-(vb,k) 128x128
    tiles; bias = bw + bz + 0.5*w_a
Device kernel (per core), W-stationary matmul with psum layout [v, rows]:
  ~26 warmup matmuls on zeroed scratch at t=0 (HAM clock ramp overlaps
  the initial DMA wait), then for each rowblock (RB=512 rows):
    for vb in 0..7:                       # 128-wide slices of V
      psum[vb] += sum_k W[vb,k].T @ xt[k]      (32 accumulating matmuls)
      t = tanh(psum + bias_vb)            # one ACT op, per-partition bias
      s_acc = (t * vwt[:,vb]) + s_acc     # fused DVE op (second stage)
    u_psum = ones.T @ s_acc               # one PE partition-reduction
    att[rowblock] = exp(u_psum)           # ACT reads psum, DMA out
  (the reduction/exp for rowblock r is emitted one mm-group into
  rowblock r+1 so the in-order PE never waits on ACT/DVE)
The device emits exp(u); the softmax normalization (one scalar divide
per batch) and the final gather happen host-side.

The vb scalar is dropped: softmax is shift-invariant.

MAIN_DT selects the matmul dtype: "bf16" (PE roofline 216ns/MM @N=512,
~1e-2 rel err) or "f32r" (fp32 data, +13.5ns/MM fixed overhead,
~1e-3 rel err). Measured on HW: bf16 N=512 back-to-back spacing 216ns
(ideal 213.3), f32r N=256 120ns (ideal 106.7).
"""

import numpy as np
import ml_dtypes

# ---------------------------------------------------------------------------
# Problem constants (hardcoded; kernel.py must be self-contained)
# ---------------------------------------------------------------------------
S, B, H2, V = 2048, 16, 2048, 1024
ALPHA_S = 0.5
NCORES = 8
BC = B // NCORES            # local batches per core
ROWS = S * BC               # 4096 rows per core (b-major)
H = 2 * H2                  # 4096 contraction dim (hidden ++ z)
P = 128
NK = H // P                 # 32
NVB = V // P                # 8

MAIN_DT = "bf16"            # "bf16" | "f32r"
RB = 512 if MAIN_DT == "bf16" else 256
NRB = ROWS // RB
XT_BUFS = 12 if MAIN_DT == "bf16" else 7
RPB = NRB // BC             # rowblocks per local batch


# ---------------------------------------------------------------------------
# Workarounds for this walrus build's 1-sync-wait-per-instruction limit
# ---------------------------------------------------------------------------
def _install_drain_patch():
    import concourse.mybir as mybir
    from concourse.tile import TileContext
    from concourse.vector_clock import ScopedClock

    def _drain_and_barrier(self, tick_clock, wait_clock):
        nc = self.nc
        drain_inst = nc.sync.drain()
        wait_clock.add_sem_waits(
            drain_inst.ins, ScopedClock({None: tick_clock.global_clock})
        )
        si = drain_inst.ins.sync_info
        if si is not None:
            waits = list(si.on_wait)
            if len(waits) > 1:
                si.on_wait = [waits[0]]
                for w in waits[1:]:
                    nop = nc.sync.nop(nofuse=True)
                    nop.ins.sync_info = mybir.SyncInfo(on_wait=[w], on_update=[])
        nc.all_engine_barrier()
        assert self.sems is not None
        popped = nc._tile_sem_poison_stack.pop()
        assert popped is self._sem_poison
        nc.clear_and_free_semaphores(list(self.sems.allocated().values()))
        nc.all_engine_barrier()

    TileContext._drain_and_barrier = _drain_and_barrier


def _split_multiwait(nc):
    """Hoist extra sync waits onto same-engine event-semaphore instructions
    inserted just before the carrying instruction."""
    import concourse.mybir as mybir

    counter = 0
    for fn in nc.m.functions:
        for bb in fn.blocks:
            insts = bb.instructions
            new_list = []
            changed = False
            for inst in insts:
                si = inst.sync_info
                if si is not None:
                    waits = list(si.on_wait)
                    if len(waits) > 1:
                        for w in waits[:-1]:
                            counter += 1
                            nop = mybir.InstEventSemaphore(
                                name=f"I-mwsplit-{counter}"
                            )
                            nop.engine = inst.engine
                            nop.bass_nofuse = True
                            nop.sync_info = mybir.SyncInfo(
                                on_wait=[w], on_update=[]
                            )
                            nc.register_instruction(nop)
                            new_list.append(nop)
                        si.on_wait = [waits[-1]]
                        changed = True
                new_list.append(inst)
            if changed:
                bb.instructions = new_list
    return counter


# ---------------------------------------------------------------------------
# Kernel build
# ---------------------------------------------------------------------------
def _build_nc():
    import concourse.bass as bass
    import concourse.mybir as mybir
    from concourse.tile import TileContext

    f32 = mybir.dt.float32
    f32r = mybir.dt.float32r
    DT = mybir.dt.bfloat16 if MAIN_DT == "bf16" else f32r

    nc = bass.Bass()
    # W pre-tiled host-side: tile (vb, k) is [P, 128] contiguous
    w_d = nc.declare_dram_parameter("w", [NVB, P, NK * P], DT, isOutput=False)
    # X^T packed host-side: xt[r, p, k, c] = X^T[k*P+p, r*RB+c]
    xt_d = nc.declare_dram_parameter("xt", [NRB, P, NK, RB], DT, isOutput=False)
    bct_d = nc.declare_dram_parameter("bct", [P, NVB], f32, isOutput=False)
    vwt_d = nc.declare_dram_parameter("vwt", [P, NVB], f32, isOutput=False)
    ones_d = nc.declare_dram_parameter("ones", [P, 1], f32r, isOutput=False)
    # att holds exp(u); the softmax normalization (one scalar divide per
    # batch) happens host-side on the gathered output
    att_d = nc.declare_dram_parameter("att", [BC, S], f32, isOutput=True)

    with TileContext(nc) as tc:
        with (
            tc.tile_pool(name="wpool", bufs=1) as wpool,
            tc.tile_pool(name="xpool", bufs=1) as xpool,
            tc.tile_pool(name="tpool", bufs=1) as tpool,
            tc.tile_pool(name="spool", bufs=1) as spool,
            tc.tile_pool(name="pspool", bufs=1, space="PSUM") as pspool,
        ):
            # --- constants: issued on the scalar engine's HWDGE so the
            # sync engine's issue bandwidth (565ns/DMA) is reserved for
            # the latency-critical xt loads
            bct_sb = spool.tile([P, NVB], f32, name="bct_sb")
            nc.scalar.dma_start(out=bct_sb[:], in_=bct_d[:, :])
            vwt_sb = spool.tile([P, NVB], f32, name="vwt_sb")
            nc.scalar.dma_start(out=vwt_sb[:], in_=vwt_d[:, :])
            ones_sb = spool.tile([P, 1], f32r, name="ones_sb")
            nc.scalar.dma_start(out=ones_sb[:], in_=ones_d[:, :])

            # --- HAM warmup: the PE clock sits at 1.2 GHz until ~3.4us of
            # sustained matmul activity. Run throwaway matmuls on zeroed
            # scratch during the initial DMA wait so the real stream is at
            # 2.4 GHz from its first instruction.
            wu_x = spool.tile([P, RB], DT, name="wu_x")
            nc.vector.memset(wu_x[:], 0.0)
            wu_ps = pspool.tile([P, RB], f32, name="wu_ps")
            for _ in range(16):
                nc.tensor.matmul(
                    wu_ps[:], wu_x[:, 0:P], wu_x[:], start=True, stop=True
                )

            # --- resident weights, loaded in k-chunks so the first
            # matmuls can start before the whole slab lands; every chunk
            # is a separate DMA -> spread across queues (per-queue DMA
            # bandwidth is only ~20 GB/s)
            w_sb = [None] * NVB

            def load_w(vb, nsplit):
                kc = NK // nsplit
                t = wpool.tile([P, NK, P], DT, name=f"w_{vb}")
                for j in range(nsplit):
                    nc.sync.dma_start(
                        out=t[:, j * kc : (j + 1) * kc],
                        in_=w_d[
                            vb, :, j * kc * P : (j + 1) * kc * P
                        ].rearrange("p (k q) -> p k q", q=P),
                    )
                w_sb[vb] = t

            # xt loaded in groups of KG k-tiles; each group tile is filled
            # by `xsplit` separate DMAs (latency: one queue moves only
            # ~20 GB/s, so a 1 MiB group would take ~50 us on one queue)
            KG = 8
            NKG = NK // KG

            def load_xt(r, xsplit=2):
                tiles = []
                for g in range(NKG):
                    t = xpool.tile(
                        [P, KG, RB], DT, name=f"xt_{r}_{g}", tag="xt",
                        bufs=XT_BUFS,
                    )
                    kc = KG // xsplit
                    for j in range(xsplit):
                        nc.sync.dma_start(
                            out=t[:, j * kc : (j + 1) * kc],
                            in_=xt_d[
                                r, :, g * KG + j * kc : g * KG + (j + 1) * kc, :
                            ],
                        )
                    tiles.append(t)
                return tiles

            # First inputs: the very first chunks are 64 KiB and
            # interleaved so the first matmul's inputs (w0 k0-1 and
            # xt g0 k0) land as early as possible.
            w0 = wpool.tile([P, NK, P], DT, name="w_0")
            w_sb[0] = w0

            def w0_chunk(k0, k1):
                nc.sync.dma_start(
                    out=w0[:, k0:k1],
                    in_=w_d[0, :, k0 * P : k1 * P].rearrange(
                        "p (k q) -> p k q", q=P
                    ),
                )

            g0 = xpool.tile(
                [P, KG, RB], DT, name="xt_0_0", tag="xt", bufs=XT_BUFS
            )
            w0_chunk(0, 2)
            nc.sync.dma_start(
                out=g0[:, 0:1, 0 : RB // 2], in_=xt_d[0, :, 0:1, 0 : RB // 2]
            )
            nc.sync.dma_start(
                out=g0[:, 0:1, RB // 2 : RB], in_=xt_d[0, :, 0:1, RB // 2 : RB]
            )
            w0_chunk(2, 4)
            for k in range(1, KG):
                nc.sync.dma_start(
                    out=g0[:, k : k + 1, :], in_=xt_d[0, :, k : k + 1, :]
                )
                w0_chunk(4 * k, 4 * k + 4)
            xt_tiles = [g0]
            for g in range(1, NKG):
                t = xpool.tile(
                    [P, KG, RB], DT, name=f"xt_0_{g}", tag="xt", bufs=XT_BUFS
                )
                for j in range(KG):
                    nc.sync.dma_start(
                        out=t[:, j : j + 1, :],
                        in_=xt_d[0, :, g * KG + j : g * KG + j + 1, :],
                    )
                xt_tiles.append(t)
            for vb in range(1, NVB):
                load_w(vb, nsplit=4)
            # rowblock 1 prefetched up front (finer split) so it cannot
            # starve behind the W slabs in the queue FIFOs
            xt_next = load_xt(1, xsplit=4) if NRB > 1 else None

            # Second stage: s_acc[p, c] = sum_vb vwt[p, vb] * tanh_vb[p, c]
            # accumulated on the DVE; one ones-reduction matmul per
            # rowblock turns that into u[c] (partition reduction), and the
            # ACT exp reads that psum directly into SBUF for the output
            # DMA. PE does only 2048 main matmuls + 8 tiny reductions.
            s_acc_of = {}
            u_ps_of = {}

            def epilogue(r):
                """Emitted one mm-group after rowblock r ends: the ones-
                reduction matmul (PE), exp (ACT), and the output DMA."""
                u_ps_of[r] = pspool.tile(
                    [1, RB], f32, name="u_ps", tag="ups", bufs=2
                )
                nc.tensor.matmul(
                    u_ps_of[r][:],
                    ones_sb[:],
                    s_acc_of[r][:],
                    start=True,
                    stop=True,
                )
                u_att = tpool.tile([1, RB], f32, name="u_att", tag="ua", bufs=2)
                nc.scalar.activation(
                    u_att[:],
                    u_ps_of[r][:],
                    mybir.ActivationFunctionType.Exp,
                )
                b = r // RPB
                s0 = (r % RPB) * RB
                nc.sync.dma_start(
                    out=att_d[b : b + 1, s0 : s0 + RB], in_=u_att[:]
                )

            pending_r = None

            for r in range(NRB):
                s_acc = tpool.tile(
                    [P, RB], f32r, name="s_acc", tag="sa", bufs=2
                )
                s_acc_of[r] = s_acc
                for vb in range(NVB):
                    ps = pspool.tile([P, RB], f32, name="ps", tag="ps", bufs=4)
                    for k in range(NK):
                        nc.tensor.matmul(
                            ps[:],
                            w_sb[vb][:, k],
                            xt_tiles[k // KG][:, k % KG],
                            start=(k == 0),
                            stop=(k == NK - 1),
                        )
                    if pending_r is not None and vb == 1:
                        epilogue(pending_r)
                        pending_r = None
                    tt = tpool.tile([P, RB], f32r, name="tt", tag="tt", bufs=3)
                    nc.scalar.activation(
                        tt[:],
                        ps[:],
                        mybir.ActivationFunctionType.Tanh,
                        bias=bct_sb[:, vb : vb + 1],
                        scale=1.0,
                    )
                    if vb == 0:
                        nc.vector.tensor_scalar_mul(
                            s_acc[:], tt[:], vwt_sb[:, 0:1]
                        )
                    else:
                        # s_acc = (tt * vwt[:, vb]) + s_acc, fused on DVE
                        nc.vector.scalar_tensor_tensor(
                            s_acc[:],
                            tt[:],
                            vwt_sb[:, vb : vb + 1],
                            s_acc[:],
                            mybir.AluOpType.mult,
                            mybir.AluOpType.add,
                        )
                pending_r = r
                if r + 1 < NRB:
                    xt_tiles = xt_next
                    xt_next = load_xt(r + 2) if r + 2 < NRB else None
            epilogue(pending_r)

    _split_multiwait(nc)
    return nc


# ---------------------------------------------------------------------------
# Host entry point
# ---------------------------------------------------------------------------
def kernel(hidden, z, Ww, bw, Wz, bz, Vw, vb, w_a):
    _install_drain_patch()
    from concourse.bass_utils import run_bass_kernel_spmd

    np_main = ml_dtypes.bfloat16 if MAIN_DT == "bf16" else np.float32

    # ---- host-side shard prep ----
    hid_t = np.ascontiguousarray(
        np.asarray(hidden).astype(np_main).transpose(2, 1, 0)
    )  # [H2, B, S]
    z_t = np.ascontiguousarray(
        np.asarray(z).astype(np_main).transpose(2, 1, 0)
    )  # [H2, B, S]

    w_cat = np.concatenate(
        [np.asarray(Ww), np.asarray(Wz)], axis=0
    ).astype(np_main)  # [H, V]
    # reorder so tile (vb) is [P, NK*P] with per-partition-contiguous rows:
    # w_r[vb, p, k*P+q] = W[k*P+p, vb*P+q]
    w_r = np.ascontiguousarray(
        w_cat.reshape(NK, P, NVB, P).transpose(2, 1, 0, 3)
    ).reshape(NVB, P, NK * P)

    bias = (
        np.asarray(bw).astype(np.float64)
        + np.asarray(bz).astype(np.float64)
        + float(np.asarray(w_a)) * ALPHA_S
    ).astype(np.float32)  # [V]
    bct = np.ascontiguousarray(bias.reshape(NVB, P).T)  # [P, NVB]
    vwt = np.ascontiguousarray(
        np.asarray(Vw).astype(np.float32).reshape(NVB, P).T
    )  # [P, NVB]

    in_maps = []
    for c in range(NCORES):
        xt_c = np.empty((H, ROWS), dtype=np_main)
        xt_c[:H2] = hid_t[:, 2 * c : 2 * c + 2, :].reshape(H2, ROWS)
        xt_c[H2:] = z_t[:, 2 * c : 2 * c + 2, :].reshape(H2, ROWS)
        # pack: xt_p[r, p, k, c] = X^T[k*P+p, r*RB+c]
        xt_p = np.ascontiguousarray(
            xt_c.reshape(NK, P, NRB, RB).transpose(2, 1, 0, 3)
        )
        in_maps.append(
            {
                "xt": xt_p,
                "w": w_r,
                "bct": bct,
                "vwt": vwt,
                "ones": np.ones((P, 1), dtype=np.float32),
            }
        )

    nc = _build_nc()
    res = run_bass_kernel_spmd(nc, in_maps, list(range(NCORES)))

    out = np.empty((S, B, 1), dtype=np.float32)
    for c in range(NCORES):
        att = res.results[c]["att"]  # [BC, S] = exp(u); normalize here
        for b in range(BC):
            e = att[b].astype(np.float64)
            out[:, 2 * c + b, 0] = (e / e.sum()).astype(np.float32)
    return out
